# revision 6
# baseline (speedup 1.0000x reference)
"""MoE block (router + top-2 expert MLPs) on 8 Trainium2 NeuronCores.

Strategy (expert-parallel):
  - Router (x @ Wr + br, top-2, softmax) computed on host with jax using the
    exact expression of the reference so expert selection matches bitwise.
  - Tokens are dispatched by expert: core e receives the tokens whose top-2
    includes expert e (padded to a fixed capacity CAP), plus expert e's
    weights W1[e]/b1[e]/W2[e]/b2[e].
  - Each core runs a Bass/Tile kernel computing
        y = sigmoid(relu(x @ W1 + b1) @ W2 + b2)
    for its CAP tokens using fp32r matmuls (full PE rate).
  - Host combines: out[t] = sum_k weight[t,k] * y_e[t].

Kernel layout per core:
  xT [D, CAP] (tokens pre-transposed on host), W1 [D, H], W2 [H, D].
  Loop over 3 token groups of 384; per group y accumulates in PSUM
  (3 x [128 tok, 1024 d] tiles = 6 banks) across all 32 h-chunks; the
  h tile (128 h x 384 tok) uses 2 more banks. First-layer matmul:
  h^T[hc] = W1[:, hc]^T-style (lhsT=W1 chunk, rhs=xT chunk); relu+b1 via
  ScalarE into SBUF; second-layer matmul: lhsT=h tile slice, rhs=W2 chunk.
  b2 is added via a rank-1 (K=1) matmul with a ones vector. A subset of
  weight chunks stays resident in SBUF to cut HBM re-streaming across
  groups.
"""

import numpy as np

D = 1024
H = 4096
E = 8
TOPK = 2
B = 4096

P = 128
KC = D // P          # 8 contraction chunks for layer 1
HC = H // P          # 32 h chunks
GROUP = 384          # tokens per PSUM-resident group
MSUB = GROUP // P    # 3 token subtiles per group
NGRP = 3             # groups per core
CAP = GROUP * NGRP   # 1152 token capacity per core
RESIDENT = 14        # weight chunks kept in SBUF across groups
N_CORES = 8

_compiled_nc = None


def _build_nc(matmul_dtype_name: str = "float32r"):
    import concourse.bacc as bacc
    import concourse.mybir as mybir
    import concourse.tile as tile

    f32 = mybir.dt.float32
    mmdt = getattr(mybir.dt, matmul_dtype_name)
    AF = mybir.ActivationFunctionType

    nc = bacc.Bacc("TRN2", target_bir_lowering=False, debug=False,
                   enable_asserts=False)

    xt_d = nc.dram_tensor("xt", (D, CAP), f32, kind="ExternalInput")
    w1_d = nc.dram_tensor("w1", (D, H), f32, kind="ExternalInput")
    b1_d = nc.dram_tensor("b1", (H,), f32, kind="ExternalInput")
    w2_d = nc.dram_tensor("w2", (H, D), f32, kind="ExternalInput")
    b2_d = nc.dram_tensor("b2", (D,), f32, kind="ExternalInput")
    ones_d = nc.dram_tensor("ones", (P,), f32, kind="ExternalInput")
    y_d = nc.dram_tensor("y", (CAP, D), f32, kind="ExternalOutput")

    # DRAM views with the partition dim (128) first.
    xt_v = xt_d.ap().rearrange("(kc p) t -> p kc t", p=P)      # [128, 8, CAP]
    w1_v = w1_d.ap().rearrange("(kc p) h -> p kc h", p=P)      # [128, 8, H]
    w2_v = w2_d.ap().rearrange("(hc p) d -> p hc d", p=P)      # [128, 32, D]
    b1_v = b1_d.ap().rearrange("(hc p) -> p hc", p=P)          # [128, 32]
    y_v = y_d.ap().rearrange("(g m p) d -> g m p d", g=NGRP, m=MSUB)

    with tile.TileContext(nc) as tc:
        with (
            tc.tile_pool(name="const", bufs=1) as cpool,
            tc.tile_pool(name="wres", bufs=1) as respool,
            tc.tile_pool(name="xg", bufs=2) as xpool,
            tc.tile_pool(name="w1s", bufs=2) as w1pool,
            tc.tile_pool(name="w2s", bufs=2) as w2pool,
            tc.tile_pool(name="hsb", bufs=2) as hpool,
            tc.tile_pool(name="yout", bufs=2) as ypool_sb,
            tc.tile_pool(name="hps", bufs=2, space="PSUM") as hpsum,
            tc.tile_pool(name="yps", bufs=1, space="PSUM") as ypsum,
        ):
            # Constants
            b1_sb = cpool.tile([P, HC], f32)
            nc.sync.dma_start(b1_sb[:], b1_v)
            b2_sb = cpool.tile([1, D], mmdt)
            nc.sync.dma_start(b2_sb[:], b2_d.ap()[None, :].bitcast(mmdt))
            ones_sb = cpool.tile([1, P], mmdt)
            nc.sync.dma_start(ones_sb[:], ones_d.ap()[None, :].bitcast(mmdt))

            # Resident weight chunks (loaded once, reused by all groups)
            w1_res = []
            w2_res = []
            for hc in range(RESIDENT):
                w1c = respool.tile([P, KC, P], mmdt, tag=f"w1r{hc}")
                nc.sync.dma_start(
                    w1c[:], w1_v[:, :, hc * P:(hc + 1) * P].bitcast(mmdt))
                w2c = respool.tile([P, D], mmdt, tag=f"w2r{hc}")
                nc.sync.dma_start(w2c[:], w2_v[:, hc, :].bitcast(mmdt))
                w1_res.append(w1c)
                w2_res.append(w2c)

            for g in range(NGRP):
                xg = xpool.tile([P, KC, GROUP], mmdt)
                nc.sync.dma_start(
                    xg[:],
                    xt_v[:, :, g * GROUP:(g + 1) * GROUP].bitcast(mmdt))

                yps = [ypsum.tile([P, D], f32, name=f"yps{m}", tag=f"yps{m}")
                       for m in range(MSUB)]

                # Initialize y accumulation with b2 (rank-1 matmul with ones)
                for m in range(MSUB):
                    for h2 in range(2):
                        nc.tensor.matmul(
                            yps[m][:, h2 * 512:(h2 + 1) * 512],
                            ones_sb[:],
                            b2_sb[:, h2 * 512:(h2 + 1) * 512],
                            start=True, stop=False,
                        )

                for hc in range(HC):
                    if hc < RESIDENT:
                        w1c = w1_res[hc]
                        w2c = w2_res[hc]
                    else:
                        w1c = w1pool.tile([P, KC, P], mmdt)
                        nc.sync.dma_start(
                            w1c[:],
                            w1_v[:, :, hc * P:(hc + 1) * P].bitcast(mmdt))
                        w2c = w2pool.tile([P, D], mmdt)
                        nc.sync.dma_start(
                            w2c[:], w2_v[:, hc, :].bitcast(mmdt))

                    # Layer 1: h^T chunk [128 h, GROUP tok]
                    hps = hpsum.tile([P, GROUP], f32)
                    for kc in range(KC):
                        nc.tensor.matmul(
                            hps[:],
                            w1c[:, kc, :],
                            xg[:, kc, :],
                            start=(kc == 0), stop=(kc == KC - 1),
                        )
                    hsb = hpool.tile([P, GROUP], mmdt)
                    nc.scalar.activation(
                        hsb[:], hps[:], AF.Relu, bias=b1_sb[:, hc:hc + 1])

                    # Layer 2: accumulate into y PSUM
                    for m in range(MSUB):
                        lhs = hsb[:, m * P:(m + 1) * P]
                        for h2 in range(2):
                            nc.tensor.matmul(
                                yps[m][:, h2 * 512:(h2 + 1) * 512],
                                lhs,
                                w2c[:, h2 * 512:(h2 + 1) * 512],
                                start=False, stop=(hc == HC - 1),
                            )

                # Epilogue: sigmoid + store
                for m in range(MSUB):
                    yo = ypool_sb.tile([P, D], f32)
                    nc.scalar.activation(yo[:], yps[m][:], AF.Sigmoid)
                    nc.sync.dma_start(y_v[g, m], yo[:])

    nc.compile()
    return nc


def _routing(x, Wr, br):
    """Router computed with the same jax expression as the reference."""
    import jax
    import jax.numpy as jnp

    logits = jnp.asarray(x) @ jnp.asarray(Wr) + jnp.asarray(br)
    topk_vals, topk_idx = jax.lax.top_k(logits, TOPK)
    weights = jax.nn.softmax(topk_vals, axis=-1)
    return np.asarray(topk_idx), np.asarray(weights, np.float32)


def _get_nc():
    global _compiled_nc
    if _compiled_nc is None:
        _compiled_nc = _build_nc()
    return _compiled_nc


def kernel(x, Wr, br, W1, b1, W2, b2, _trace=False, _trace_kwargs=None):
    from concourse import bass_utils

    x = np.ascontiguousarray(np.asarray(x, dtype=np.float32))
    Wr = np.asarray(Wr, dtype=np.float32)
    br = np.asarray(br, dtype=np.float32)
    W1 = np.asarray(W1, dtype=np.float32)
    b1 = np.asarray(b1, dtype=np.float32)
    W2 = np.asarray(W2, dtype=np.float32)
    b2 = np.asarray(b2, dtype=np.float32)

    topk_idx, wts = _routing(x, Wr, br)

    # Per-expert token lists and weights
    tok_lists = []
    wt_lists = []
    for e in range(E):
        mask = topk_idx == e                      # [B, TOPK]
        toks = np.nonzero(mask.any(axis=1))[0]
        # weight of expert e for each selected token (exactly one slot matches)
        slot = mask[toks].argmax(axis=1)
        tok_lists.append(toks)
        wt_lists.append(wts[toks, slot])

    nc = _get_nc()

    out = np.zeros((B, D), dtype=np.float32)
    max_count = max(len(t) for t in tok_lists)
    n_waves = max(1, -(-max_count // CAP))
    last_result = None
    for wave in range(n_waves):
        in_maps = []
        for e in range(E):
            toks = tok_lists[e][wave * CAP:(wave + 1) * CAP]
            xt = np.zeros((D, CAP), dtype=np.float32)
            if len(toks):
                xt[:, :len(toks)] = x[toks].T
            in_maps.append({
                "xt": xt,
                "ones": np.ones((P,), dtype=np.float32),
                "w1": np.ascontiguousarray(W1[e]),
                "b1": np.ascontiguousarray(b1[e]),
                "w2": np.ascontiguousarray(W2[e]),
                "b2": np.ascontiguousarray(b2[e]),
            })
        res = bass_utils.run_bass_kernel_spmd(
            nc, in_maps, core_ids=list(range(N_CORES)),
            trace=_trace, **(_trace_kwargs or {}))
        last_result = res
        for e in range(E):
            toks = tok_lists[e][wave * CAP:(wave + 1) * CAP]
            if len(toks) == 0:
                continue
            y_e = res.results[e]["y"][:len(toks)]
            out[toks] += wt_lists[e][wave * CAP:(wave + 1) * CAP][:, None] * y_e

    if _trace:
        kernel.last_result = last_result
    return out


# revision 7
# speedup vs baseline: 1.6417x; 1.6417x over previous
"""MoE block (router + top-2 expert MLPs) on 8 Trainium2 NeuronCores.

Strategy (expert-parallel):
  - Router (x @ Wr + br, top-2, softmax) computed on host with jax using the
    exact expression of the reference so expert selection matches bitwise.
  - Tokens are dispatched by expert: core e receives the tokens whose top-2
    includes expert e (padded to a fixed capacity CAP), plus expert e's
    weights W1[e]/b1[e]/W2[e]/b2[e].
  - Each core runs a Bass/Tile kernel computing
        y = sigmoid(relu(x @ W1 + b1) @ W2 + b2)
    for its CAP tokens with fp16 matmuls (fp32 PSUM accumulation; fp16
    streams 2 elements per 4-byte SBUF read so the PE runs at 1 col/cycle
    vs 2 cycles/col for fp32/fp32r operands).
  - Host combines: out[t] = sum_k weight[t,k] * y_e[t].

Kernel layout per core:
  xT [D, CAP] fp16 (tokens gathered+transposed+converted on host),
  W1 [D, H] fp16, W2 [H, D] fp16, b1 fp32, b2 fp16.
  All 32 weight h-chunks are SBUF-resident (fp16 halves the footprint), so
  weights stream from HBM exactly once. Loop over 3 token groups of 384;
  per group y accumulates in PSUM (3 x [128 tok, 1024 d] fp32 tiles =
  6 banks) across all 32 h-chunks; the h tile (128 h x 384 tok) uses 2
  more banks. Layer 1: lhsT=W1 chunk, rhs=xT chunk -> h^T in PSUM;
  relu+b1 via ScalarE into fp16 SBUF; layer 2: lhsT=h tile slice,
  rhs=W2 chunk, accumulating into the y PSUM tiles. b2 is pre-added via a
  rank-1 (K=1) matmul with a ones vector; sigmoid+store per group.
"""

import numpy as np

D = 1024
H = 4096
E = 8
TOPK = 2
B = 4096

P = 128
KC = D // P          # 8 contraction chunks for layer 1
HC = H // P          # 32 h chunks
GROUP = 384          # tokens per PSUM-resident group
MSUB = GROUP // P    # 3 token subtiles per group
NGRP = 3             # groups per core
CAP = GROUP * NGRP   # 1152 token capacity per core
N_CORES = 8

_compiled_nc = None


def _build_nc(mm_dtype_name: str = "float16"):
    import concourse.bacc as bacc
    import concourse.mybir as mybir
    import concourse.tile as tile

    f32 = mybir.dt.float32
    mmdt = getattr(mybir.dt, mm_dtype_name)
    AF = mybir.ActivationFunctionType

    nc = bacc.Bacc("TRN2", target_bir_lowering=False, debug=False,
                   enable_asserts=False)

    xt_d = nc.dram_tensor("xt", (D, CAP), mmdt, kind="ExternalInput")
    w1_d = nc.dram_tensor("w1", (D, H), mmdt, kind="ExternalInput")
    b1_d = nc.dram_tensor("b1", (H,), f32, kind="ExternalInput")
    w2_d = nc.dram_tensor("w2", (H, D), mmdt, kind="ExternalInput")
    b2_d = nc.dram_tensor("b2", (D,), mmdt, kind="ExternalInput")
    ones_d = nc.dram_tensor("ones", (P,), mmdt, kind="ExternalInput")
    y_d = nc.dram_tensor("y", (CAP, D), f32, kind="ExternalOutput")

    # DRAM views with the partition dim (128) first.
    xt_v = xt_d.ap().rearrange("(kc p) t -> p kc t", p=P)      # [128, 8, CAP]
    w1_v = w1_d.ap().rearrange("(kc p) h -> p kc h", p=P)      # [128, 8, H]
    w2_v = w2_d.ap().rearrange("(hc p) d -> p hc d", p=P)      # [128, 32, D]
    b1_v = b1_d.ap().rearrange("(hc p) -> p hc", p=P)          # [128, 32]
    y_v = y_d.ap().rearrange("(g m p) d -> g m p d", g=NGRP, m=MSUB)

    with tile.TileContext(nc) as tc:
        with (
            tc.tile_pool(name="const", bufs=1) as cpool,
            tc.tile_pool(name="wres", bufs=1) as respool,
            tc.tile_pool(name="hsb", bufs=2) as hpool,
            tc.tile_pool(name="yout", bufs=2) as ypool_sb,
            tc.tile_pool(name="hps", bufs=2, space="PSUM") as hpsum,
            tc.tile_pool(name="yps", bufs=1, space="PSUM") as ypsum,
        ):
            # Constants
            b1_sb = cpool.tile([P, HC], f32)
            nc.sync.dma_start(b1_sb[:], b1_v)
            b2_sb = cpool.tile([1, D], mmdt)
            nc.sync.dma_start(b2_sb[:], b2_d.ap()[None, :])
            ones_sb = cpool.tile([1, P], mmdt)
            nc.sync.dma_start(ones_sb[:], ones_d.ap()[None, :])
            x_sb = cpool.tile([P, KC, CAP], mmdt)
            for kc in range(KC):
                nc.sync.dma_start(x_sb[:, kc, :], xt_v[:, kc, :])

            # All weight chunks SBUF-resident (streamed from HBM once)
            w1_res = []
            w2_res = []
            for hc in range(HC):
                w1c = respool.tile([P, KC, P], mmdt, tag=f"w1r{hc}")
                nc.sync.dma_start(w1c[:], w1_v[:, :, hc * P:(hc + 1) * P])
                w2c = respool.tile([P, D], mmdt, tag=f"w2r{hc}")
                nc.sync.dma_start(w2c[:], w2_v[:, hc, :])
                w1_res.append(w1c)
                w2_res.append(w2c)

            for g in range(NGRP):
                yps = [ypsum.tile([P, D], f32, name=f"yps{m}", tag=f"yps{m}")
                       for m in range(MSUB)]

                # Initialize y accumulation with b2 (rank-1 matmul with ones)
                for m in range(MSUB):
                    for h2 in range(2):
                        nc.tensor.matmul(
                            yps[m][:, h2 * 512:(h2 + 1) * 512],
                            ones_sb[:],
                            b2_sb[:, h2 * 512:(h2 + 1) * 512],
                            start=True, stop=False,
                        )

                for hc in range(HC):
                    w1c = w1_res[hc]
                    w2c = w2_res[hc]

                    # Layer 1: h^T chunk [128 h, GROUP tok]
                    hps = hpsum.tile([P, GROUP], f32)
                    for kc in range(KC):
                        nc.tensor.matmul(
                            hps[:],
                            w1c[:, kc, :],
                            x_sb[:, kc, g * GROUP:(g + 1) * GROUP],
                            start=(kc == 0), stop=(kc == KC - 1),
                        )
                    hsb = hpool.tile([P, GROUP], mmdt)
                    nc.scalar.activation(
                        hsb[:], hps[:], AF.Relu, bias=b1_sb[:, hc:hc + 1])

                    # Layer 2: accumulate into y PSUM
                    for m in range(MSUB):
                        lhs = hsb[:, m * P:(m + 1) * P]
                        for h2 in range(2):
                            nc.tensor.matmul(
                                yps[m][:, h2 * 512:(h2 + 1) * 512],
                                lhs,
                                w2c[:, h2 * 512:(h2 + 1) * 512],
                                start=False, stop=(hc == HC - 1),
                            )

                # Epilogue: sigmoid + store
                for m in range(MSUB):
                    yo = ypool_sb.tile([P, D], f32)
                    nc.scalar.activation(yo[:], yps[m][:], AF.Sigmoid)
                    nc.sync.dma_start(y_v[g, m], yo[:])

    nc.compile()
    return nc


def _routing(x, Wr, br):
    """Router computed with the same jax expression as the reference."""
    import jax
    import jax.numpy as jnp

    logits = jnp.asarray(x) @ jnp.asarray(Wr) + jnp.asarray(br)
    topk_vals, topk_idx = jax.lax.top_k(logits, TOPK)
    weights = jax.nn.softmax(topk_vals, axis=-1)
    return np.asarray(topk_idx), np.asarray(weights, np.float32)


def _get_nc():
    global _compiled_nc
    if _compiled_nc is None:
        _compiled_nc = _build_nc()
    return _compiled_nc


def kernel(x, Wr, br, W1, b1, W2, b2, _trace=False, _trace_kwargs=None):
    from concourse import bass_utils

    x = np.ascontiguousarray(np.asarray(x, dtype=np.float32))
    Wr = np.asarray(Wr, dtype=np.float32)
    br = np.asarray(br, dtype=np.float32)
    W1 = np.asarray(W1, dtype=np.float32)
    b1 = np.asarray(b1, dtype=np.float32)
    W2 = np.asarray(W2, dtype=np.float32)
    b2 = np.asarray(b2, dtype=np.float32)

    topk_idx, wts = _routing(x, Wr, br)

    # Per-expert token lists and weights
    tok_lists = []
    wt_lists = []
    for e in range(E):
        mask = topk_idx == e                      # [B, TOPK]
        toks = np.nonzero(mask.any(axis=1))[0]
        # weight of expert e for each selected token (exactly one slot matches)
        slot = mask[toks].argmax(axis=1)
        tok_lists.append(toks)
        wt_lists.append(wts[toks, slot])

    nc = _get_nc()

    xh = x.astype(np.float16)
    W1h = W1.astype(np.float16)
    W2h = W2.astype(np.float16)
    b2h = b2.astype(np.float16)

    out = np.zeros((B, D), dtype=np.float32)
    max_count = max(len(t) for t in tok_lists)
    n_waves = max(1, -(-max_count // CAP))
    last_result = None
    for wave in range(n_waves):
        in_maps = []
        for e in range(E):
            toks = tok_lists[e][wave * CAP:(wave + 1) * CAP]
            xt = np.zeros((D, CAP), dtype=np.float16)
            if len(toks):
                xt[:, :len(toks)] = xh[toks].T
            in_maps.append({
                "xt": xt,
                "ones": np.ones((P,), dtype=np.float16),
                "w1": np.ascontiguousarray(W1h[e]),
                "b1": np.ascontiguousarray(b1[e]),
                "w2": np.ascontiguousarray(W2h[e]),
                "b2": np.ascontiguousarray(b2h[e]),
            })
        res = bass_utils.run_bass_kernel_spmd(
            nc, in_maps, core_ids=list(range(N_CORES)),
            trace=_trace, **(_trace_kwargs or {}))
        last_result = res
        for e in range(E):
            toks = tok_lists[e][wave * CAP:(wave + 1) * CAP]
            if len(toks) == 0:
                continue
            y_e = res.results[e]["y"][:len(toks)]
            out[toks] += wt_lists[e][wave * CAP:(wave + 1) * CAP][:, None] * y_e

    if _trace:
        kernel.last_result = last_result
    return out
